# revision 73
# baseline (speedup 1.0000x reference)
"""Trainium2 Bass kernel: batched recursive Newton-Euler inverse dynamics
(7-dof serial chain) — data-parallel over 8 NeuronCores.

Per core, the 65536-row shard lives as planes [128 part, 512 free].
Per-link parameters are baked in as immediate constants. The physics is
emitted through a symbolic layer (Val = a*plane + c) that prunes zeros and
folds scales. Linear combinations are emitted SCALE-FREE: per-term 1-src
scale ops (tensor_scalar, 148 ns in f16 4x mode) + pure tensor_tensor
add/sub chains (327 ns in f16 2x mode) — cheaper on the DVE than fused
594 ns scalar_tensor_tensor chains, which get no f16 fast path. All planes
are fp16 except the trig range-reduction chain (fp32 magic-constant
rounding); inputs are cast once. Ops are recorded into a tiny IR, DCE'd,
and list-scheduled (liveness-pressure- and window-bounded) before emission;
temporaries go to SBUF "registers" via linear-scan liveness with FIFO
same-engine reuse (Tile pool slot rotation is strict round-robin, so naive
tmp pools deadlock; eager cross-engine reuse creates WAR semaphores).

Engine placement (HW-raced): DVE does nearly everything; ACT gets the Sin
activations plus at most ACT_MAX_OPS=500 scale affines (HW races: 200-500 ACT
ops help ~70 us, but the ~1700-edge fine-grained 3-engine split TimelineSim
prefers is SLOWER than DVE-only on hardware — cross-engine semaphore
traffic costs far more than the model's 260 ns; Pool ops also cost
806-1111 ns vs DVE 148-594 and lose in every race).
"""

import math
from contextlib import ExitStack

import numpy as np

P = 128
D = 7
N_CORES = 8
BATCH = 524288
SHARD = BATCH // N_CORES      # 65536
FD = SHARD // P               # 512

SCALE_FREE_LIN = True   # lin chains as scale-op + tensor_tensor (f16-fast)
                        # vs fused scalar_tensor_tensor (fewer instructions)


# ---------------------------------------------------------------------------
# symbolic value: a * plane + c   (plane None -> pure constant)
# ---------------------------------------------------------------------------
class Val:
    __slots__ = ("pl", "a", "c")

    def __init__(self, pl, a=1.0, c=0.0):
        self.pl = pl
        self.a = float(a)
        self.c = float(c)
        if pl is None:
            self.a = 0.0

    @property
    def is_const(self):
        return self.pl is None or self.a == 0.0


def VC(c):
    return Val(None, 0.0, c)


class Builder:
    """Backend-agnostic emitter. Each primitive is exactly one instruction."""

    def __init__(self):
        self.n_2src = 0
        self.n_1src = 0
        self.n_trig = 0
        self.phase = ""
        self._ones = None

    # ---- primitives (backends) ----
    def p_stt(self, in0, scalar, in1, op1, dest=None):
        raise NotImplementedError

    def p_tt(self, in0, in1, op, dest=None):
        raise NotImplementedError

    def p_affine(self, in0, scale, bias, dest=None):
        raise NotImplementedError

    def p_sin(self, in0, scale, bias):
        raise NotImplementedError

    def p_sinact(self, in0, scale, bias):
        """bare Sin activation: sin(scale*in0 + bias), |arg| <= pi."""
        raise NotImplementedError

    def p_ones(self):
        raise NotImplementedError

    def inp(self, name, j):
        raise NotImplementedError

    def inpc(self, name, j):
        """input column cast to the working (possibly f16) dtype."""
        return self.inp(name, j)

    def out_ap(self, j):
        raise NotImplementedError

    def f_ap(self, j, i):
        raise NotImplementedError

    def state_ap(self, j, i):
        raise NotImplementedError

    def plane_key(self, pl):
        return id(pl)

    def same_plane(self, a, b):
        return a is b

    # ---- helpers ----
    def ones(self):
        if self._ones is None:
            self._ones = self.p_ones()
        return self._ones

    CACHE_SINCOS = False
    CACHE_SCALES = False

    def sincos(self, j):
        # shared fp32 range reduction (one per joint): r = round(x/2pi) via
        # the magic-constant trick (the +MAGIC fold into the first affine is
        # exact only because the phase bias is 0); z = x - 2pi*r in [-pi,pi].
        # s = Sin(z) directly (in spline range); c via half-angle
        # c = 1 - 2*sin(z/2)^2 since Sin(z + pi/2) would leave the range.
        self.n_trig += 2
        x = self.inp("q", j)
        TWO_PI = 2.0 * math.pi
        MAGIC = 12582912.0  # 1.5 * 2**23
        u2 = self.p_affine(x, 1.0 / TWO_PI, MAGIC)
        u3 = self.p_affine(u2, 1.0, -MAGIC)
        z = self.p_stt(u3, -TWO_PI, x, "add")
        self.mark_f32((u2, u3, z))
        # z is reduced to [-pi, pi]: Sin(z) is directly in spline range;
        # only cos needs the half-angle form (Sin(z + pi/2) would not be)
        spl = self.p_sinact(z, 1.0, 0.0)
        sh = self.p_sinact(z, 0.5, 0.0)
        self.n_2src += 1
        sq = self.p_tt(sh, sh, "mult")
        self.n_1src += 1
        cpl = self.p_affine(sq, -2.0, 1.0)
        return Val(spl, 1.0, 0.0), Val(cpl, 1.0, 0.0)

    def mark_f32(self, toks):
        pass

    def scaled(self, pl, r):
        """memoized r*plane (shared across lin chains)."""
        if not self.CACHE_SCALES:
            self.n_1src += 1
            return self.p_affine(pl, r, 0.0)
        if not hasattr(self, "_scale_cache"):
            self._scale_cache = {}
        key = (self.plane_key(pl), float(r))
        if key not in self._scale_cache:
            self.n_1src += 1
            self._scale_cache[key] = self.p_affine(pl, r, 0.0)
        return self._scale_cache[key]

    def lin(self, vals, coefs, const=0.0, dest=None, exact=False, scale_free=False):
        terms = {}
        c_acc = float(const)
        for v, k in zip(vals, coefs):
            k = float(k)
            if k == 0.0:
                continue
            c_acc += k * v.c
            if v.pl is not None and v.a != 0.0:
                key = self.plane_key(v.pl)
                if key in terms:
                    terms[key][1] += k * v.a
                else:
                    terms[key] = [v.pl, k * v.a]
        tl = [(pl, k) for pl, k in terms.values() if k != 0.0]
        if not tl:
            if dest is not None:
                self.n_1src += 1
                self.p_affine(self.ones(), c_acc, 0.0, dest=dest)
                return Val(dest, 1.0, 0.0)
            return VC(c_acc)
        if c_acc != 0.0:
            tl.append((self.ones(), c_acc))
        if len(tl) == 1:
            pl, k = tl[0]
            if dest is not None:
                self.n_1src += 1
                self.p_affine(pl, k, 0.0, dest=dest)
                return Val(dest, 1.0, 0.0)
            if exact and k != 1.0:
                self.n_1src += 1
                return Val(self.p_affine(pl, k, 0.0), 1.0, 0.0)
            return Val(pl, k, 0.0)
        if not SCALE_FREE_LIN:
            # fused form: n-1 scalar_tensor_tensor ops (no scale ops, fewer
            # instructions, but no f16 fast mode on DVE)
            tl.sort(key=lambda t: abs(t[1]))
            cur_pl, cur_k = tl[0]
            for i in range(1, len(tl)):
                pl_i, k_i = tl[i]
                is_last = i == len(tl) - 1
                use_dest = dest is not None and is_last and (scale_free
                                                            or k_i == 1.0)
                dd = dest if use_dest else None
                self.n_2src += 1
                cur_pl = self.p_stt(cur_pl, cur_k / k_i, pl_i, "add", dest=dd)
                cur_k = k_i
            if dest is not None and not self.same_plane(cur_pl, dest):
                self.n_1src += 1
                self.p_affine(cur_pl, cur_k, 0.0, dest=dest)
                return Val(dest, 1.0, 0.0)
            if dest is not None:
                return Val(dest, cur_k if scale_free else 1.0, 0.0)
            if exact and cur_k != 1.0:
                self.n_1src += 1
                return Val(self.p_affine(cur_pl, cur_k, 0.0), 1.0, 0.0)
            return Val(cur_pl, cur_k, 0.0)
        # scale-free chain: pick a base term, pre-scale mismatched terms with
        # independent 1-src affine ops (cheap / off the bottleneck engine),
        # then combine with pure tensor_tensor add/sub (fast in f16 on DVE).
        from collections import Counter

        cnt = Counter(abs(k) for _, k in tl)
        want_unit = dest is not None and not scale_free
        bi = max(range(len(tl)),
                 key=lambda i: (cnt[abs(tl[i][1])],
                                1 if (want_unit and tl[i][1] == 1.0) else 0,
                                -i))
        maxk = max(abs(k) for _, k in tl)
        if maxk / abs(tl[bi][1]) > 16.0:
            # keep pre-scaled terms in f16 range: ratios must stay modest
            bi = max(range(len(tl)), key=lambda i: abs(tl[i][1]))
        pl_b, k_b = tl[bi]
        rest = [tl[i] for i in range(len(tl)) if i != bi]
        rest.sort(key=lambda t: abs(t[1]))
        cur_pl, cur_k = pl_b, k_b
        for i, (pl_i, k_i) in enumerate(rest):
            r = k_i / k_b
            is_last = i == len(rest) - 1
            use_dest = dest is not None and is_last and (scale_free
                                                        or k_b == 1.0)
            d = dest if use_dest else None
            if r == 1.0:
                self.n_2src += 1
                cur_pl = self.p_tt(cur_pl, pl_i, "add", dest=d)
            elif r == -1.0:
                self.n_2src += 1
                cur_pl = self.p_tt(cur_pl, pl_i, "subtract", dest=d)
            else:
                sc = self.scaled(pl_i, r)
                self.n_2src += 1
                cur_pl = self.p_tt(cur_pl, sc, "add", dest=d)
        if dest is not None and not self.same_plane(cur_pl, dest):
            self.n_1src += 1
            self.p_affine(cur_pl, cur_k, 0.0, dest=dest)
            return Val(dest, 1.0, 0.0)
        if dest is not None:
            return Val(dest, cur_k if scale_free else 1.0, 0.0)
        if exact and cur_k != 1.0:
            self.n_1src += 1
            return Val(self.p_affine(cur_pl, cur_k, 0.0), 1.0, 0.0)
        return Val(cur_pl, cur_k, 0.0)

    def mov(self, v, dest):
        self.n_1src += 1
        if v.pl is None:
            self.p_affine(self.ones(), v.c, 0.0, dest=dest)
        else:
            self.p_affine(v.pl, v.a, v.c, dest=dest)
        return Val(dest, 1.0, 0.0)

    def mul(self, x, y):
        if x.is_const and y.is_const:
            return VC(x.c * y.c)
        if x.is_const:
            x, y = y, x
        if y.is_const:
            return Val(x.pl, x.a * y.c, x.c * y.c)
        xp, yp = x, y
        if xp.c != 0.0:
            self.n_1src += 1
            xp = Val(self.p_affine(xp.pl, 1.0, xp.c / xp.a), xp.a, 0.0)
        if yp.c != 0.0:
            self.n_1src += 1
            yp = Val(self.p_affine(yp.pl, 1.0, yp.c / yp.a), yp.a, 0.0)
        self.n_2src += 1
        out = self.p_tt(xp.pl, yp.pl, "mult")
        return Val(out, xp.a * yp.a, 0.0)

    def cross(self, u, v):
        out = []
        for i in range(3):
            b, c = (i + 1) % 3, (i + 2) % 3
            m1 = self.mul(u[b], v[c])
            m2 = self.mul(u[c], v[b])
            out.append((m1, m2))
        return out

    def matvec(self, M, v, exact=True):
        # exact=True normalizes output scales to 1: downstream givens /
        # cross / force chains then combine with pure +-1 coefficients
        # (tensor_tensor only), instead of fragmenting into scale ops.
        return [self.lin(v, [M[i][0], M[i][1], M[i][2]], exact=exact)
                for i in range(3)]

    def givens(self, c, s, k, sgn, w, inverse, dests=None):
        a, b = (k + 1) % 3, (k + 2) % 3
        sg = -sgn if inverse else sgn
        out = [None, None, None]
        if w[a].is_const and w[b].is_const:
            out[a] = self.lin([c, s], [w[a].c, -sg * w[b].c])
            out[b] = self.lin([s, c], [sg * w[a].c, w[b].c])
        else:
            ca = self.mul(c, w[a])
            cb = self.mul(c, w[b])
            sa = self.mul(s, w[a])
            sb = self.mul(s, w[b])
            da = dests[a] if dests else None
            db = dests[b] if dests else None
            out[a] = self.lin([ca, sb], [1.0, -sg], dest=da, scale_free=True)
            out[b] = self.lin([sa, cb], [sg, 1.0], dest=db, scale_free=True)
        out[k] = w[k]
        if dests:
            if dests[a] is not None and (out[a].pl is None
                                         or not self.same_plane(out[a].pl, dests[a])):
                out[a] = self.mov(out[a], dests[a])
            if dests[b] is not None and (out[b].pl is None
                                         or not self.same_plane(out[b].pl, dests[b])):
                out[b] = self.mov(out[b], dests[b])
            if dests[k] is not None and not w[k].is_const:
                out[k] = self.mov(w[k], dests[k])
        return out


# ---------------------------------------------------------------------------
# host-side constants
# ---------------------------------------------------------------------------
def host_consts(rot_fix, trans_fix, joint_axes, mass, com, inertia, damping):
    rot_fix = np.asarray(rot_fix, np.float64)
    trans_fix = np.asarray(trans_fix, np.float64)
    joint_axes = np.asarray(joint_axes, np.float64)
    mass = np.asarray(mass, np.float64)
    com = np.asarray(com, np.float64)
    inertia = np.asarray(inertia, np.float64)
    damping = np.asarray(damping, np.float64)
    C = {}
    C["F"] = [rot_fix[j + 1] for j in range(D)]
    C["p"] = [trans_fix[j + 1] for j in range(D)]
    ax = []
    for j in range(D):
        k = int(np.argmax(np.abs(joint_axes[j])))
        ax.append((k, float(np.sign(joint_axes[j][k]))))
    C["ax"] = ax
    C["m"] = [float(mass[j + 1]) for j in range(D)]
    C["mc"] = [mass[j + 1] * com[j + 1] for j in range(D)]
    Io = []
    for j in range(D):
        cc = com[j + 1]
        cs = np.array([[0, -cc[2], cc[1]], [cc[2], 0, -cc[0]], [-cc[1], cc[0], 0]])
        Io.append(inertia[j + 1] + mass[j + 1] * (cs @ cs.T))
    C["Io"] = Io
    C["damping"] = [float(damping[j]) for j in range(D)]
    C["G"] = 9.81
    return C


# ---------------------------------------------------------------------------
# the physics graph (backend-independent)
# ---------------------------------------------------------------------------
def build_rnea(b: Builder, C):
    Z = VC(0.0)
    vl = [Z, Z, Z]
    va = [Z, Z, Z]
    al = [Z, Z, VC(C["G"])]
    aa = [Z, Z, Z]
    fstore = [[None] * 6 for _ in range(D)]
    for j in range(D):
        F = C["F"][j]
        p = C["p"][j]
        k, sg = C["ax"][j]
        a_, b_ = (k + 1) % 3, (k + 2) % 3
        b.phase = f"fwd{j}"
        s, c = b.sincos(j)
        qd = Val(b.inpc("qd", j))
        qdd = Val(b.inpc("qdd", j))
        Ft = F.T.tolist()

        def dvec(x, y):
            out = []
            for i in range(3):
                bb, cc = (i + 1) % 3, (i + 2) % 3
                out.append(b.lin([x[i], y[cc], y[bb]], [1.0, -p[bb], p[cc]]))
            return out

        u_vl = b.matvec(Ft, dvec(vl, va))
        u_va = b.matvec(Ft, va)
        u_al = b.matvec(Ft, dvec(al, aa))
        u_aa = b.matvec(Ft, aa)
        std = lambda i: b.state_ap(j, i)
        vl_i = b.givens(c, s, k, sg, u_vl, True, dests=[std(0), std(1), std(2)])
        va_r = b.givens(c, s, k, sg, u_va, True,
                        dests=[std(3 + i) if i != k else None for i in range(3)])
        va_i = list(va_r)
        va_i[k] = b.lin([va_r[k], qd], [1.0, sg], dest=std(3 + k), scale_free=True)
        al_r = b.givens(c, s, k, sg, u_al, True,
                        dests=[std(6 + i) if i == k else None for i in range(3)])
        aa_r = b.givens(c, s, k, sg, u_aa, True)
        aa_i = list(aa_r)
        aa_i[k] = b.lin([aa_r[k], qdd], [1.0, sg], dest=std(9 + k), scale_free=True)
        ek = [0.0, 0.0, 0.0]
        ek[k] = 1.0
        al_i = list(al_r)
        for i in (a_, b_):
            bb, cc = (i + 1) % 3, (i + 2) % 3
            cva = b.lin([va_i[bb], va_i[cc]], [ek[cc], -ek[bb]])
            m1 = b.mul(cva, qd)
            aa_i[i] = b.lin([aa_r[i], m1], [1.0, sg], dest=std(9 + i),
                            scale_free=True)
            cvl = b.lin([vl_i[bb], vl_i[cc]], [ek[cc], -ek[bb]])
            m2 = b.mul(cvl, qd)
            al_i[i] = b.lin([al_r[i], m2], [1.0, sg], dest=std(6 + i),
                            scale_free=True)
        vl, va, al, aa = vl_i, va_i, al_i, aa_i

        # ---- force for this joint ----
        b.phase = f"force{j}"
        m = C["m"][j]
        mc = C["mc"][j].tolist()
        Io = C["Io"][j]
        Iv_l = [b.lin([vl[i], va[(i + 1) % 3], va[(i + 2) % 3]],
                      [m, mc[(i + 2) % 3], -mc[(i + 1) % 3]]) for i in range(3)]
        Ia_l = [b.lin([al[i], aa[(i + 1) % 3], aa[(i + 2) % 3]],
                      [m, mc[(i + 2) % 3], -mc[(i + 1) % 3]]) for i in range(3)]
        Iv_a = [b.lin([va[0], va[1], va[2], vl[(i + 2) % 3], vl[(i + 1) % 3]],
                      [Io[i][0], Io[i][1], Io[i][2],
                       mc[(i + 1) % 3], -mc[(i + 2) % 3]]) for i in range(3)]
        Ia_a = [b.lin([aa[0], aa[1], aa[2], al[(i + 2) % 3], al[(i + 1) % 3]],
                      [Io[i][0], Io[i][1], Io[i][2],
                       mc[(i + 1) % 3], -mc[(i + 2) % 3]]) for i in range(3)]
        cv1 = b.cross(va, Iv_l)
        for i in range(3):
            m1, m2 = cv1[i]
            fstore[j][i] = b.lin([Ia_l[i], m1, m2], [1.0, 1.0, -1.0],
                                 dest=b.f_ap(j, i), scale_free=True)
        cv2 = b.cross(va, Iv_a)
        cv3 = b.cross(vl, Iv_l)
        for i in range(3):
            m1, m2 = cv2[i]
            m3, m4 = cv3[i]
            fstore[j][3 + i] = b.lin([Ia_a[i], m1, m2, m3, m4],
                                     [1.0, 1.0, -1.0, 1.0, -1.0],
                                     dest=b.f_ap(j, 3 + i), scale_free=True)

    # ---- backward pass ----
    cl = [Z, Z, Z]
    ca = [Z, Z, Z]
    for j in range(D - 1, -1, -1):
        F = C["F"][j]
        p = C["p"][j]
        k, sg = C["ax"][j]
        pp = (F.T @ p).tolist()
        b.phase = f"bwd{j}"
        s, c = b.sincos(j)
        f_l = fstore[j][:3]
        f_a = fstore[j][3:]
        tl = [b.lin([f_l[i], cl[i]], [1.0, 1.0]) for i in range(3)]
        ta = [b.lin([f_a[i], ca[i]], [1.0, 1.0]) for i in range(3)]
        b.lin([ta[k], Val(b.inpc("qd", j))], [sg, C["damping"][j]],
              dest=b.out_ap(j))
        if j == 0:
            continue
        w_l = b.givens(c, s, k, sg, tl, False)
        w_a = b.givens(c, s, k, sg, ta, False)
        x = []
        for i in range(3):
            bb, cc = (i + 1) % 3, (i + 2) % 3
            x.append(b.lin([w_a[i], w_l[cc], w_l[bb]], [1.0, pp[bb], -pp[cc]]))
        cl = b.matvec(F.tolist(), w_l)
        ca = b.matvec(F.tolist(), x)


# ---------------------------------------------------------------------------
# numpy backend (validation)
# ---------------------------------------------------------------------------
class NumpyBuilder(Builder):
    def __init__(self, q, qd, qdd, f16=False):
        super().__init__()
        self.q, self.qd, self.qdd = q, qd, qdd
        self.N = q.shape[0]
        self.out = np.zeros((self.N, D), np.float32)
        self._f = {}
        self.f16 = f16

    def _w(self, r, dest):
        if self.f16:
            r = r.astype(np.float16).astype(np.float32)
        if dest is not None:
            dest[...] = r
            return dest
        return r

    def _f32(self, x):
        return np.asarray(x, np.float32)

    def p_stt(self, in0, scalar, in1, op1, dest=None):
        r = self._f32(in0 * np.float32(scalar))
        if op1 == "add":
            r = self._f32(r + in1)
        elif op1 == "subtract":
            r = self._f32(r - in1)
        else:
            r = self._f32(r * in1)
        return self._w(r, dest)

    def p_tt(self, in0, in1, op, dest=None):
        if op == "mult":
            r = self._f32(in0 * in1)
        elif op == "add":
            r = self._f32(in0 + in1)
        else:
            r = self._f32(in0 - in1)
        return self._w(r, dest)

    def p_affine(self, in0, scale, bias, dest=None):
        return self._w(self._f32(in0 * np.float32(scale) + np.float32(bias)), dest)

    def p_sinact(self, in0, scale, bias):
        r = self._f32(np.sin(self._f32(in0 * np.float32(scale) + np.float32(bias))))
        if self.f16:
            r = r.astype(np.float16).astype(np.float32)
        return r

    def p_ones(self):
        return np.ones(self.N, np.float32)

    def inp(self, name, j):
        return {"q": self.q, "qd": self.qd, "qdd": self.qdd}[name][:, j].astype(
            np.float32
        )

    def inpc(self, name, j):
        v = self.inp(name, j)
        if self.f16:
            v = v.astype(np.float16).astype(np.float32)
        return v

    def sincos(self, j):
        # numeric equivalent of Builder.sincos with the fp32 reduction kept
        # unrounded (the magic constant overflows a naive f16 emulation)
        self.n_trig += 2
        x = self.inp("q", j)
        TWO_PI = np.float32(2.0 * math.pi)
        r = np.round((x / TWO_PI).astype(np.float32))
        z = (x - TWO_PI * r).astype(np.float32)

        def rnd(v):
            v = np.asarray(v, np.float32)
            return v.astype(np.float16).astype(np.float32) if self.f16 else v

        spl = rnd(np.sin(z))
        sh = rnd(np.sin(np.float32(0.5) * z))
        sq = rnd(sh * sh)
        cpl = rnd(np.float32(1.0) - np.float32(2.0) * sq)
        return Val(spl, 1.0, 0.0), Val(cpl, 1.0, 0.0)

    def out_ap(self, j):
        return self.out[:, j]

    def f_ap(self, j, i):
        key = (j, i)
        if key not in self._f:
            self._f[key] = np.empty(self.N, np.float32)
        return self._f[key]

    def state_ap(self, j, i):
        return np.empty(self.N, np.float32)


def rnea_numpy(q, qd, qdd, rot_fix, trans_fix, joint_axes, mass, com, inertia,
               damping):
    C = host_consts(rot_fix, trans_fix, joint_axes, mass, com, inertia, damping)
    b = NumpyBuilder(q, qd, qdd)
    build_rnea(b, C)
    return b.out


# ---------------------------------------------------------------------------
# IR backend: records ops on integer-token planes
# ---------------------------------------------------------------------------
class IRBuilder(Builder):
    def __init__(self):
        super().__init__()
        self.ops = []   # (kind, out_token, in_tokens, params, phase)
        self._n = 0
        self.phase = ""
        self.f32_toks = set()   # tmp tokens that must stay fp32 (trig chain)
        self._casts = {}

    def _tmp(self):
        self._n += 1
        return ("t", self._n)

    def plane_key(self, pl):
        return pl

    def same_plane(self, a, b):
        return a == b

    def p_stt(self, in0, scalar, in1, op1, dest=None):
        out = dest if dest is not None else self._tmp()
        self.ops.append(("stt", out, (in0, in1), (float(scalar), op1),
                         self.phase))
        return out

    def p_tt(self, in0, in1, op, dest=None):
        out = dest if dest is not None else self._tmp()
        self.ops.append(("tt", out, (in0, in1), (op,), self.phase))
        return out

    def p_affine(self, in0, scale, bias, dest=None):
        out = dest if dest is not None else self._tmp()
        self.ops.append(("affine", out, (in0,), (float(scale), float(bias)),
                         self.phase))
        return out

    def p_sinact(self, in0, scale, bias):
        out = self._tmp()
        self.ops.append(("sinact", out, (in0,), (float(bias), float(scale)),
                         self.phase))
        return out

    def mark_f32(self, toks):
        self.f32_toks.update(toks)

    def p_ones(self):
        out = ("ones",)
        self.ops.append(("memset", out, (), (1.0,), self.phase))
        return out

    def inp(self, name, j):
        return ("in", name, j)

    def inpc(self, name, j):
        key = (name, j)
        if key not in self._casts:
            self._casts[key] = self.p_affine(("in", name, j), 1.0, 0.0)
        return self._casts[key]

    def out_ap(self, j):
        return ("out", j)

    def f_ap(self, j, i):
        return ("f", j, i)

    def state_ap(self, j, i):
        return self._tmp()


def dce(ops):
    """drop ops whose results are never used (named 'out'/'f' sinks are live;
    'f' only if read)."""
    needed = set()
    keep = [False] * len(ops)
    for idx in range(len(ops) - 1, -1, -1):
        kind, out, ins, params, phase = ops[idx]
        if out[0] == "out" or out in needed:
            keep[idx] = True
            for t in ins:
                needed.add(t)
    return [op for k2, op in zip(keep, ops) if k2]


def ir_stats(ops):
    from collections import Counter

    c = Counter(k for k, *_ in ops)
    # liveness: peak concurrent tmp planes
    last_use = {}
    for idx, (kind, out, ins, params, phase) in enumerate(ops):
        for t in ins:
            if t[0] == "t":
                last_use[t] = idx
    live = set()
    peak = 0
    for idx, (kind, out, ins, params, phase) in enumerate(ops):
        if out[0] == "t":
            live.add(out)
        peak = max(peak, len(live))
        for t in ins:
            if t[0] == "t" and last_use.get(t) == idx:
                live.discard(t)
    return dict(c), peak


# ---------------------------------------------------------------------------
# engine assignment: HEFT-style list scheduling over DVE / Pool / ACT
# ---------------------------------------------------------------------------
# measured per-op costs on [128, 512] tiles (TimelineSim == HW +-4%).
# HW ISA facts: Pool supports tensor_tensor / tensor_scalar-imm / copy but NOT
# scalar_tensor_tensor; DVE f16 fast modes need every operand 2-byte.
ENGINE_FILTER = {"DVE", "ACT"}   # HW-raced: DVE + a capped ACT dose;
                                 # POOL and uncapped ACT lose on HW
ACT_TRIG_ONLY = False  # restrict ACT to the trig pipeline (plus sinact)
ACT_MAX_OPS = 500      # cap on ACT-assigned affine ops (edge-dose control;
                       # HW dose-response flat 200-500, sim prefers more)
POOL_PHASES = ()       # phases owned exclusively by the Pool engine
XENG_NS = 260.0       # cross-engine dependency (semaphore) latency
EDGE_COST_NS = 0.0    # busy-time surcharge per cross-engine input (makes the
                      # greedy actively avoid creating semaphore edges)
RAW_BUBBLE_NS = 120.0  # extra delay when consumer directly follows producer
                      # on the same engine (SBUF write->read turnaround)
SCHED_WINDOW = 360    # max scheduler runahead past the frontier
SCHED_PRESSURE = 56   # live-tmp count above which pressure relief kicks in


def schedule_engines(ops, f32_toks=frozenset(), dtype16=True):
    """Assign each op an engine and a global emission order. Returns
    (ordered ops, engine per ordered op, est. makespan ns)."""

    def is16(t):
        if t[0] == "t":
            return dtype16 and t not in f32_toks
        if t[0] == "f":
            return True
        if t[0] == "ones":
            return dtype16
        return False   # in / out dram-backed chunks are fp32

    def op_costs(i):
        kind, out, ins, params, phase = ops[i]
        fast = dtype16 and is16(out) and all(is16(t) for t in ins)
        if kind == "stt":
            c = {"DVE": 594.0}
        elif kind == "tt":
            c = {"DVE": 327.0 if fast else 594.0, "POOL": 1111.0}
            if phase in POOL_PHASES:
                return {"POOL": 1111.0}
        elif kind == "affine":
            c = {"DVE": 148.0 if fast else 297.5, "ACT": 612.0,
                 "POOL": 806.0}
            if phase in POOL_PHASES:
                return {"POOL": 806.0}
            if ACT_TRIG_ONLY and out not in f32_toks:
                c.pop("ACT")
        elif kind == "sinact":
            return {"ACT": 612.0}
        elif kind == "memset":
            return {"DVE": 297.5}
        else:
            raise ValueError(kind)
        c2 = {e: v for e, v in c.items() if e in ENGINE_FILTER}
        return c2 or c

    n = len(ops)
    prod = {}
    for i, (kind, out, ins, params, phase) in enumerate(ops):
        prod[out] = i
    preds = [[] for _ in range(n)]
    succs = [[] for _ in range(n)]
    for i, (kind, out, ins, params, phase) in enumerate(ops):
        seen = set()
        for t in ins:
            j = prod.get(t)
            if j is not None and j < i and j not in seen:
                seen.add(j)
                preds[i].append(j)
                succs[j].append(i)
    costs = [op_costs(i) for i in range(n)]
    mincost = [min(costs[i].values()) for i in range(n)]
    rank = [0.0] * n
    for i in range(n - 1, -1, -1):
        m = 0.0
        for s in succs[i]:
            if rank[s] > m:
                m = rank[s]
        rank[i] = mincost[i] + m
    # register pressure bookkeeping: how many pending reads of each tmp
    uses = {}
    for i, (kind, out, ins, params, phase) in enumerate(ops):
        for t in ins:
            if t[0] == "t":
                uses[t] = uses.get(t, 0) + 1

    def presses(i):
        """net change in live tmp count if op i runs now."""
        kind, out, ins, params, phase = ops[i]
        d = 1 if (out[0] == "t") else 0
        for t in set(ins):
            if t[0] == "t" and uses[t] == sum(1 for x in ins if x == t):
                d -= 1
        return d

    ndeps = [len(preds[i]) for i in range(n)]
    ready = [i for i in range(n) if ndeps[i] == 0]
    eng_free = {"DVE": 0.0, "POOL": 0.0, "ACT": 0.0}
    eng_last = {"DVE": -1, "POOL": -1, "ACT": -1}   # last op on each engine
    act_used = 0
    finish = [0.0] * n
    start = [0.0] * n
    eng_of = [None] * n
    order = []
    live = 0
    done = 0
    scheduled = [False] * n
    frontier = 0          # lowest unscheduled original index
    WINDOW = SCHED_WINDOW
    PRESSURE_HI = SCHED_PRESSURE
    while ready:
        while frontier < n and scheduled[frontier]:
            frontier += 1
        elig = [x for x in ready if x < frontier + WINDOW]
        if not elig:
            elig = ready
        if live > PRESSURE_HI:
            i = min(elig, key=lambda x: (presses(x), -rank[x]))
        else:
            i = max(elig, key=lambda x: rank[x])
        ready.remove(i)
        scheduled[i] = True
        kind = ops[i][0]
        best = None
        cand = costs[i]
        if kind == "affine" and act_used >= ACT_MAX_OPS and "ACT" in cand \
                and len(cand) > 1:
            cand = {e: v for e, v in cand.items() if e != "ACT"}
        for e, c in cand.items():
            est = eng_free[e]
            n_cross = 0
            for p in preds[i]:
                if eng_of[p] != e:
                    d = finish[p] + XENG_NS
                    n_cross += 1
                elif p == eng_last[e]:
                    d = finish[p] + RAW_BUBBLE_NS
                else:
                    d = finish[p]
                if d > est:
                    est = d
            f = est + c + EDGE_COST_NS * n_cross
            if best is None or f < best[0] or (f == best[0] and c < best[1]):
                best = (f, c, e, est)
        f, c, e, st = best
        eng_of[i] = e
        start[i] = st
        finish[i] = f
        eng_free[e] = f
        eng_last[e] = i
        if e == "ACT" and kind == "affine":
            act_used += 1
        done += 1
        kind_i, out_i, ins_i, _, _ = ops[i]
        if out_i[0] == "t":
            live += 1
        for t in ins_i:
            if t[0] == "t":
                uses[t] -= 1
                if uses[t] == 0:
                    live -= 1
        for s in succs[i]:
            ndeps[s] -= 1
            if ndeps[s] == 0:
                ready.append(s)
        order.append(i)
    assert done == n
    return ([ops[i] for i in order], [eng_of[i] for i in order],
            max(finish) if n else 0.0)


def build_ir(C):
    b = IRBuilder()
    build_rnea(b, C)
    ops = dce(b.ops)
    return ops, b


# ---------------------------------------------------------------------------
# bass emission from IR
# ---------------------------------------------------------------------------
def emit_bass(nc, tc, pools, chunks, out_chunk, ops, engines, f32_toks,
              fd=FD, bench_alias_out=False, dtype16=True, fstore16=True):
    from concourse import mybir

    f32 = mybir.dt.float32
    fdt = mybir.dt.float16 if dtype16 else mybir.dt.float32
    fsdt = mybir.dt.float16 if (fstore16 or dtype16) else mybir.dt.float32
    ALU = {"add": mybir.AluOpType.add, "subtract": mybir.AluOpType.subtract,
           "mult": mybir.AluOpType.mult}

    def tok_dt(tok):
        if tok[0] == "t" and tok in f32_toks:
            return f32
        return fdt

    last_use = {}
    for idx, (kind, out, ins, params, phase) in enumerate(ops):
        for t in ins:
            if t[0] == "t":
                last_use[t] = idx

    from collections import deque

    ftiles = {}
    tmp_ap = {}         # token -> AP
    reg_of = {}         # token -> (dtype key, reg index)
    free_regs = {"f32": deque(), "f16": deque()}  # FIFO: max reuse distance
    n_regs = {"f32": 0, "f16": 0}
    serial = 0

    def named_ap(tok):
        nonlocal serial
        if tok[0] == "in":
            _, name, j = tok
            v = chunks[name].rearrange("p (f d) -> p d f", d=D)
            return v[:, j, :]
        if tok[0] == "out":
            base = chunks["qdd"] if bench_alias_out else out_chunk
            v = base.rearrange("p (f d) -> p d f", d=D)
            return v[:, tok[1], :]
        if tok[0] == "f":
            _, j, i = tok
            if j not in ftiles:
                serial += 1
                ftiles[j] = pools["fst"].tile([P, 6 * fd], fsdt, tag=f"f{j}",
                                              name=f"f{j}", bufs=1)
            t = ftiles[j]
            return t[:, i * fd:(i + 1) * fd]
        if tok[0] == "ones":
            return ones_ap
        raise KeyError(tok)

    def get_ap(tok):
        if tok[0] == "t":
            return tmp_ap[tok]
        return named_ap(tok)

    def alloc_out(tok, idx, eng):
        nonlocal serial
        if tok[0] != "t":
            return named_ap(tok)
        dt = tok_dt(tok)
        key = "f32" if dt == f32 else "f16"
        fr = free_regs[key]
        r = None
        if fr:
            # prefer a register whose WAR predecessor (last reader) ran on
            # the same engine: the hazard is then free (program order)
            for k in range(len(fr)):
                if fr[k][1] == eng:
                    r = fr[k][0]
                    del fr[k]
                    break
            if r is None:
                r = fr.popleft()[0]
        else:
            r = n_regs[key]
            n_regs[key] += 1
        reg_of[tok] = (key, r)
        serial += 1
        t = pools["reg"].tile([P, fd], dt, tag=f"r{key}_{r}",
                              name=f"v{serial}", bufs=1)
        tmp_ap[tok] = t[:, :]
        return tmp_ap[tok]

    def release_ins(ins, idx, eng):
        for t in ins:
            if t[0] == "t" and last_use.get(t) == idx:
                kr = reg_of.pop(t, None)
                if kr is not None:
                    free_regs[kr[0]].append((kr[1], eng))

    ones_ap = None
    n_eng = {"DVE": 0, "POOL": 0, "ACT": 0}
    Copy = mybir.ActivationFunctionType.Copy
    for idx, (kind, out, ins, params, phase) in enumerate(ops):
        eng = engines[idx]
        n_eng[eng] += 1
        if kind == "memset":
            serial += 1
            t = pools["misc"].tile([P, fd], fdt, tag="ones", name="ones", bufs=1)
            ones_ap = t[:, :]
            nc.vector.memset(ones_ap, 1.0)
            continue
        out_ap = alloc_out(out, idx, eng)
        if kind == "stt":
            scalar, op1 = params
            assert eng == "DVE"   # Pool has no scalar_tensor_tensor on TRN2
            nc.vector.scalar_tensor_tensor(out_ap, get_ap(ins[0]), scalar,
                                           get_ap(ins[1]),
                                           mybir.AluOpType.mult, ALU[op1])
        elif kind == "tt":
            e = nc.vector if eng == "DVE" else nc.gpsimd
            e.tensor_tensor(out_ap, get_ap(ins[0]), get_ap(ins[1]),
                            ALU[params[0]])
        elif kind == "affine":
            scale, bias = params
            if eng == "ACT":
                nc.scalar.activation(out_ap, get_ap(ins[0]), Copy,
                                     bias=bias, scale=scale)
            elif eng == "DVE":
                nc.vector.tensor_scalar(out_ap, get_ap(ins[0]), scale, bias,
                                        mybir.AluOpType.mult,
                                        mybir.AluOpType.add)
            else:
                nc.gpsimd.tensor_scalar(out_ap, get_ap(ins[0]), scale, bias,
                                        mybir.AluOpType.mult,
                                        mybir.AluOpType.add)
        elif kind == "sinact":
            bias, scale = params
            nc.scalar.activation(out_ap, get_ap(ins[0]),
                                 mybir.ActivationFunctionType.Sin,
                                 bias=bias, scale=scale)
        else:
            raise ValueError(kind)
        release_ins(ins, idx, eng)
    return n_regs, n_eng


def _build_nc(C, verbose=False, repeat=1, dtype16=True):
    import concourse.bacc as bacc
    import concourse.tile as tile_mod
    from concourse import mybir

    ops, bstat = build_ir(C)
    f32_toks = bstat.f32_toks
    ops, engines, est = schedule_engines(ops, f32_toks=f32_toks,
                                         dtype16=dtype16)
    if verbose:
        stats, peak = ir_stats(ops)
        print("IR ops:", stats, "peak live tmps:", peak)
        print("schedule est makespan: %.0f ns" % est)

    nc = bacc.Bacc()
    f32 = mybir.dt.float32
    # register pi/2 as a const AP (Sin activation bias must be a const AP)
    halfpi = float(math.pi / 2)
    _ct = nc.alloc_sbuf_tensor("const-f32-halfpi", [128, 1], f32)
    nc.gpsimd.memset(_ct.ap(), halfpi)
    nc.const_aps.aps[(f32, halfpi)] = _ct.ap()
    nc.all_engine_barrier()
    q_d = nc.dram_tensor("q", [SHARD, D], f32, kind="ExternalInput")
    qd_d = nc.dram_tensor("qd", [SHARD, D], f32, kind="ExternalInput")
    qdd_d = nc.dram_tensor("qdd", [SHARD, D], f32, kind="ExternalInput")
    tau_d = nc.dram_tensor("tau", [SHARD, D], f32, kind="ExternalOutput")

    with ExitStack() as ctx:
        tc = ctx.enter_context(tile_mod.TileContext(nc))
        io_pool = ctx.enter_context(tc.tile_pool(name="io", bufs=1))
        fst_pool = ctx.enter_context(tc.tile_pool(name="fst", bufs=1))
        reg_pool = ctx.enter_context(tc.tile_pool(name="reg", bufs=1))
        misc_pool = ctx.enter_context(tc.tile_pool(name="misc", bufs=1))
        pools = {"io": io_pool, "fst": fst_pool, "reg": reg_pool,
                 "misc": misc_pool}

        chunks = {}
        for name, dram in (("q", q_d), ("qd", qd_d), ("qdd", qdd_d)):
            t = io_pool.tile([P, D * FD], f32, tag=f"io_{name}",
                             name=f"ch_{name}", bufs=1)
            nc.sync.dma_start(t[:, :],
                              dram[:, :].rearrange("(p f) d -> p (f d)", p=P))
            chunks[name] = t

        if repeat == 1:
            # out chunk shares the qdd slot (qdd is fully consumed by the
            # forward pass before any tau is written)
            out_chunk = io_pool.tile([P, D * FD], f32, tag="io_qdd",
                                     name="ch_out", bufs=1)
            n_regs, n_eng = emit_bass(nc, tc, pools, chunks, out_chunk, ops,
                                      engines, f32_toks, dtype16=dtype16)
        else:
            # bench mode: dedicated out tile (aliasing qdd would serialize
            # body k+1's forward pass behind body k's tau writes)
            out_chunk = io_pool.tile([P, D * FD], f32, tag="io_out",
                                     name="ch_out", bufs=1)
            for _ in range(repeat):
                n_regs, n_eng = emit_bass(nc, tc, pools, chunks, out_chunk,
                                          ops, engines, f32_toks,
                                          dtype16=dtype16)
        if verbose:
            print("registers used:", n_regs, "engine mix:", n_eng)

        nc.sync.dma_start(tau_d[:, :].rearrange("(p f) d -> p (f d)", p=P),
                          out_chunk[:, :])
    if not nc.is_finalized():
        nc.finalize()
    return nc


def kernel(**inputs):
    q = np.ascontiguousarray(inputs["q"], np.float32)
    qd = np.ascontiguousarray(inputs["qd"], np.float32)
    qdd = np.ascontiguousarray(inputs["qdd_des"], np.float32)
    C = host_consts(inputs["rot_fix"], inputs["trans_fix"], inputs["joint_axes"],
                    inputs["mass"], inputs["com"], inputs["inertia"],
                    inputs["damping"])
    nc = _build_nc(C)

    from concourse.bass_utils import run_bass_kernel_spmd

    in_maps = []
    for i in range(N_CORES):
        sl = slice(i * SHARD, (i + 1) * SHARD)
        in_maps.append({"q": q[sl], "qd": qd[sl], "qdd": qdd[sl]})
    res = run_bass_kernel_spmd(nc, in_maps, list(range(N_CORES)))
    out = np.concatenate([res.results[i]["tau"] for i in range(N_CORES)], 0)
    return out.astype(np.float32)



# revision 75
# speedup vs baseline: 1.1266x; 1.1266x over previous
"""Trainium2 Bass kernel: batched recursive Newton-Euler inverse dynamics
(7-dof serial chain) — data-parallel over 8 NeuronCores.

Per core, the 65536-row shard lives as planes [128 part, 512 free].
Per-link parameters are baked in as immediate constants. The physics is
emitted through a symbolic layer (Val = a*plane + c) that prunes zeros and
folds scales. Linear combinations are emitted SCALE-FREE: per-term 1-src
scale ops (tensor_scalar, 148 ns in f16 4x mode) + pure tensor_tensor
add/sub chains (327 ns in f16 2x mode) — cheaper on the DVE than fused
594 ns scalar_tensor_tensor chains, which get no f16 fast path. All planes
are fp16 except the trig range-reduction chain (fp32 magic-constant
rounding); inputs are cast once. Ops are recorded into a tiny IR, DCE'd,
and list-scheduled (liveness-pressure- and window-bounded) before emission;
temporaries go to SBUF "registers" via linear-scan liveness with FIFO
same-engine reuse (Tile pool slot rotation is strict round-robin, so naive
tmp pools deadlock; eager cross-engine reuse creates WAR semaphores).

Engine placement (HW-raced): DVE does nearly everything; ACT gets the Sin
activations plus at most ACT_MAX_OPS=500 scale affines (HW races: 200-500 ACT
ops help ~70 us, but the ~1700-edge fine-grained 3-engine split TimelineSim
prefers is SLOWER than DVE-only on hardware — cross-engine semaphore
traffic costs far more than the model's 260 ns; Pool ops also cost
806-1111 ns vs DVE 148-594 and lose in every race).
"""

import math
from contextlib import ExitStack

import numpy as np

P = 128
D = 7
N_CORES = 8
BATCH = 524288
SHARD = BATCH // N_CORES      # 65536
FD = SHARD // P               # 512

SCALE_FREE_LIN = True   # lin chains as scale-op + tensor_tensor (f16-fast)
                        # vs fused scalar_tensor_tensor (fewer instructions)


# ---------------------------------------------------------------------------
# symbolic value: a * plane + c   (plane None -> pure constant)
# ---------------------------------------------------------------------------
class Val:
    __slots__ = ("pl", "a", "c")

    def __init__(self, pl, a=1.0, c=0.0):
        self.pl = pl
        self.a = float(a)
        self.c = float(c)
        if pl is None:
            self.a = 0.0

    @property
    def is_const(self):
        return self.pl is None or self.a == 0.0


def VC(c):
    return Val(None, 0.0, c)


class Builder:
    """Backend-agnostic emitter. Each primitive is exactly one instruction."""

    def __init__(self):
        self.n_2src = 0
        self.n_1src = 0
        self.n_trig = 0
        self.phase = ""
        self._ones = None

    # ---- primitives (backends) ----
    def p_stt(self, in0, scalar, in1, op1, dest=None):
        raise NotImplementedError

    def p_tt(self, in0, in1, op, dest=None):
        raise NotImplementedError

    def p_affine(self, in0, scale, bias, dest=None):
        raise NotImplementedError

    def p_sin(self, in0, scale, bias):
        raise NotImplementedError

    def p_sinact(self, in0, scale, bias):
        """bare Sin activation: sin(scale*in0 + bias), |arg| <= pi."""
        raise NotImplementedError

    def p_ones(self):
        raise NotImplementedError

    def inp(self, name, j):
        raise NotImplementedError

    def inpc(self, name, j):
        """input column cast to the working (possibly f16) dtype."""
        return self.inp(name, j)

    def out_ap(self, j):
        raise NotImplementedError

    def f_ap(self, j, i):
        raise NotImplementedError

    def state_ap(self, j, i):
        raise NotImplementedError

    def plane_key(self, pl):
        return id(pl)

    def same_plane(self, a, b):
        return a is b

    # ---- helpers ----
    def ones(self):
        if self._ones is None:
            self._ones = self.p_ones()
        return self._ones

    CACHE_SINCOS = False
    CACHE_SCALES = False

    def sincos(self, j):
        # shared fp32 range reduction (one per joint): r = round(x/2pi) via
        # the magic-constant trick (the +MAGIC fold into the first affine is
        # exact only because the phase bias is 0); z = x - 2pi*r in [-pi,pi].
        # s = Sin(z) directly (in spline range); c via half-angle
        # c = 1 - 2*sin(z/2)^2 since Sin(z + pi/2) would leave the range.
        self.n_trig += 2
        x = self.inp("q", j)
        TWO_PI = 2.0 * math.pi
        MAGIC = 12582912.0  # 1.5 * 2**23
        u2 = self.p_affine(x, 1.0 / TWO_PI, MAGIC)
        u3 = self.p_affine(u2, 1.0, -MAGIC)
        z = self.p_stt(u3, -TWO_PI, x, "add")
        self.mark_f32((u2, u3, z))
        # z is reduced to [-pi, pi]: Sin(z) is directly in spline range;
        # only cos needs the half-angle form (Sin(z + pi/2) would not be)
        spl = self.p_sinact(z, 1.0, 0.0)
        sh = self.p_sinact(z, 0.5, 0.0)
        self.n_2src += 1
        sq = self.p_tt(sh, sh, "mult")
        self.n_1src += 1
        cpl = self.p_affine(sq, -2.0, 1.0)
        return Val(spl, 1.0, 0.0), Val(cpl, 1.0, 0.0)

    def mark_f32(self, toks):
        pass

    def scaled(self, pl, r):
        """memoized r*plane (shared across lin chains)."""
        if not self.CACHE_SCALES:
            self.n_1src += 1
            return self.p_affine(pl, r, 0.0)
        if not hasattr(self, "_scale_cache"):
            self._scale_cache = {}
        key = (self.plane_key(pl), float(r))
        if key not in self._scale_cache:
            self.n_1src += 1
            self._scale_cache[key] = self.p_affine(pl, r, 0.0)
        return self._scale_cache[key]

    def lin(self, vals, coefs, const=0.0, dest=None, exact=False, scale_free=False):
        terms = {}
        c_acc = float(const)
        for v, k in zip(vals, coefs):
            k = float(k)
            if k == 0.0:
                continue
            c_acc += k * v.c
            if v.pl is not None and v.a != 0.0:
                key = self.plane_key(v.pl)
                if key in terms:
                    terms[key][1] += k * v.a
                else:
                    terms[key] = [v.pl, k * v.a]
        tl = [(pl, k) for pl, k in terms.values() if k != 0.0]
        if not tl:
            if dest is not None:
                self.n_1src += 1
                self.p_affine(self.ones(), c_acc, 0.0, dest=dest)
                return Val(dest, 1.0, 0.0)
            return VC(c_acc)
        if c_acc != 0.0:
            tl.append((self.ones(), c_acc))
        if len(tl) == 1:
            pl, k = tl[0]
            if dest is not None:
                self.n_1src += 1
                self.p_affine(pl, k, 0.0, dest=dest)
                return Val(dest, 1.0, 0.0)
            if exact and k != 1.0:
                self.n_1src += 1
                return Val(self.p_affine(pl, k, 0.0), 1.0, 0.0)
            return Val(pl, k, 0.0)
        if not SCALE_FREE_LIN:
            # fused form: n-1 scalar_tensor_tensor ops (no scale ops, fewer
            # instructions, but no f16 fast mode on DVE)
            tl.sort(key=lambda t: abs(t[1]))
            cur_pl, cur_k = tl[0]
            for i in range(1, len(tl)):
                pl_i, k_i = tl[i]
                is_last = i == len(tl) - 1
                use_dest = dest is not None and is_last and (scale_free
                                                            or k_i == 1.0)
                dd = dest if use_dest else None
                self.n_2src += 1
                cur_pl = self.p_stt(cur_pl, cur_k / k_i, pl_i, "add", dest=dd)
                cur_k = k_i
            if dest is not None and not self.same_plane(cur_pl, dest):
                self.n_1src += 1
                self.p_affine(cur_pl, cur_k, 0.0, dest=dest)
                return Val(dest, 1.0, 0.0)
            if dest is not None:
                return Val(dest, cur_k if scale_free else 1.0, 0.0)
            if exact and cur_k != 1.0:
                self.n_1src += 1
                return Val(self.p_affine(cur_pl, cur_k, 0.0), 1.0, 0.0)
            return Val(cur_pl, cur_k, 0.0)
        # scale-free chain: pick a base term, pre-scale mismatched terms with
        # independent 1-src affine ops (cheap / off the bottleneck engine),
        # then combine with pure tensor_tensor add/sub (fast in f16 on DVE).
        from collections import Counter

        cnt = Counter(abs(k) for _, k in tl)
        want_unit = dest is not None and not scale_free
        bi = max(range(len(tl)),
                 key=lambda i: (cnt[abs(tl[i][1])],
                                1 if (want_unit and tl[i][1] == 1.0) else 0,
                                -i))
        maxk = max(abs(k) for _, k in tl)
        if maxk / abs(tl[bi][1]) > 16.0:
            # keep pre-scaled terms in f16 range: ratios must stay modest
            bi = max(range(len(tl)), key=lambda i: abs(tl[i][1]))
        pl_b, k_b = tl[bi]
        rest = [tl[i] for i in range(len(tl)) if i != bi]
        rest.sort(key=lambda t: abs(t[1]))
        cur_pl, cur_k = pl_b, k_b
        for i, (pl_i, k_i) in enumerate(rest):
            r = k_i / k_b
            is_last = i == len(rest) - 1
            use_dest = dest is not None and is_last and (scale_free
                                                        or k_b == 1.0)
            d = dest if use_dest else None
            if r == 1.0:
                self.n_2src += 1
                cur_pl = self.p_tt(cur_pl, pl_i, "add", dest=d)
            elif r == -1.0:
                self.n_2src += 1
                cur_pl = self.p_tt(cur_pl, pl_i, "subtract", dest=d)
            else:
                sc = self.scaled(pl_i, r)
                self.n_2src += 1
                cur_pl = self.p_tt(cur_pl, sc, "add", dest=d)
        if dest is not None and not self.same_plane(cur_pl, dest):
            self.n_1src += 1
            self.p_affine(cur_pl, cur_k, 0.0, dest=dest)
            return Val(dest, 1.0, 0.0)
        if dest is not None:
            return Val(dest, cur_k if scale_free else 1.0, 0.0)
        if exact and cur_k != 1.0:
            self.n_1src += 1
            return Val(self.p_affine(cur_pl, cur_k, 0.0), 1.0, 0.0)
        return Val(cur_pl, cur_k, 0.0)

    def mov(self, v, dest):
        self.n_1src += 1
        if v.pl is None:
            self.p_affine(self.ones(), v.c, 0.0, dest=dest)
        else:
            self.p_affine(v.pl, v.a, v.c, dest=dest)
        return Val(dest, 1.0, 0.0)

    def mul(self, x, y):
        if x.is_const and y.is_const:
            return VC(x.c * y.c)
        if x.is_const:
            x, y = y, x
        if y.is_const:
            return Val(x.pl, x.a * y.c, x.c * y.c)
        xp, yp = x, y
        if xp.c != 0.0:
            self.n_1src += 1
            xp = Val(self.p_affine(xp.pl, 1.0, xp.c / xp.a), xp.a, 0.0)
        if yp.c != 0.0:
            self.n_1src += 1
            yp = Val(self.p_affine(yp.pl, 1.0, yp.c / yp.a), yp.a, 0.0)
        self.n_2src += 1
        out = self.p_tt(xp.pl, yp.pl, "mult")
        return Val(out, xp.a * yp.a, 0.0)

    def cross(self, u, v):
        out = []
        for i in range(3):
            b, c = (i + 1) % 3, (i + 2) % 3
            m1 = self.mul(u[b], v[c])
            m2 = self.mul(u[c], v[b])
            out.append((m1, m2))
        return out

    def matvec(self, M, v, exact=True):
        # exact=True normalizes output scales to 1: downstream givens /
        # cross / force chains then combine with pure +-1 coefficients
        # (tensor_tensor only), instead of fragmenting into scale ops.
        return [self.lin(v, [M[i][0], M[i][1], M[i][2]], exact=exact)
                for i in range(3)]

    def givens(self, c, s, k, sgn, w, inverse, dests=None):
        a, b = (k + 1) % 3, (k + 2) % 3
        sg = -sgn if inverse else sgn
        out = [None, None, None]
        if w[a].is_const and w[b].is_const:
            out[a] = self.lin([c, s], [w[a].c, -sg * w[b].c])
            out[b] = self.lin([s, c], [sg * w[a].c, w[b].c])
        else:
            ca = self.mul(c, w[a])
            cb = self.mul(c, w[b])
            sa = self.mul(s, w[a])
            sb = self.mul(s, w[b])
            da = dests[a] if dests else None
            db = dests[b] if dests else None
            out[a] = self.lin([ca, sb], [1.0, -sg], dest=da, scale_free=True)
            out[b] = self.lin([sa, cb], [sg, 1.0], dest=db, scale_free=True)
        out[k] = w[k]
        if dests:
            if dests[a] is not None and (out[a].pl is None
                                         or not self.same_plane(out[a].pl, dests[a])):
                out[a] = self.mov(out[a], dests[a])
            if dests[b] is not None and (out[b].pl is None
                                         or not self.same_plane(out[b].pl, dests[b])):
                out[b] = self.mov(out[b], dests[b])
            if dests[k] is not None and not w[k].is_const:
                out[k] = self.mov(w[k], dests[k])
        return out


# ---------------------------------------------------------------------------
# host-side constants
# ---------------------------------------------------------------------------
def host_consts(rot_fix, trans_fix, joint_axes, mass, com, inertia, damping):
    rot_fix = np.asarray(rot_fix, np.float64)
    trans_fix = np.asarray(trans_fix, np.float64)
    joint_axes = np.asarray(joint_axes, np.float64)
    mass = np.asarray(mass, np.float64)
    com = np.asarray(com, np.float64)
    inertia = np.asarray(inertia, np.float64)
    damping = np.asarray(damping, np.float64)
    C = {}
    C["F"] = [rot_fix[j + 1] for j in range(D)]
    C["p"] = [trans_fix[j + 1] for j in range(D)]
    ax = []
    for j in range(D):
        k = int(np.argmax(np.abs(joint_axes[j])))
        ax.append((k, float(np.sign(joint_axes[j][k]))))
    C["ax"] = ax
    C["m"] = [float(mass[j + 1]) for j in range(D)]
    C["mc"] = [mass[j + 1] * com[j + 1] for j in range(D)]
    Io = []
    for j in range(D):
        cc = com[j + 1]
        cs = np.array([[0, -cc[2], cc[1]], [cc[2], 0, -cc[0]], [-cc[1], cc[0], 0]])
        Io.append(inertia[j + 1] + mass[j + 1] * (cs @ cs.T))
    C["Io"] = Io
    C["damping"] = [float(damping[j]) for j in range(D)]
    C["G"] = 9.81
    return C


# ---------------------------------------------------------------------------
# the physics graph (backend-independent)
# ---------------------------------------------------------------------------
def build_rnea(b: Builder, C):
    Z = VC(0.0)
    vl = [Z, Z, Z]
    va = [Z, Z, Z]
    al = [Z, Z, VC(C["G"])]
    aa = [Z, Z, Z]
    fstore = [[None] * 6 for _ in range(D)]
    for j in range(D):
        F = C["F"][j]
        p = C["p"][j]
        k, sg = C["ax"][j]
        a_, b_ = (k + 1) % 3, (k + 2) % 3
        b.phase = f"fwd{j}"
        s, c = b.sincos(j)
        qd = Val(b.inpc("qd", j))
        qdd = Val(b.inpc("qdd", j))
        Ft = F.T.tolist()

        def dvec(x, y):
            out = []
            for i in range(3):
                bb, cc = (i + 1) % 3, (i + 2) % 3
                out.append(b.lin([x[i], y[cc], y[bb]], [1.0, -p[bb], p[cc]]))
            return out

        u_vl = b.matvec(Ft, dvec(vl, va))
        u_va = b.matvec(Ft, va)
        u_al = b.matvec(Ft, dvec(al, aa))
        u_aa = b.matvec(Ft, aa)
        std = lambda i: b.state_ap(j, i)
        vl_i = b.givens(c, s, k, sg, u_vl, True, dests=[std(0), std(1), std(2)])
        va_r = b.givens(c, s, k, sg, u_va, True,
                        dests=[std(3 + i) if i != k else None for i in range(3)])
        va_i = list(va_r)
        va_i[k] = b.lin([va_r[k], qd], [1.0, sg], dest=std(3 + k), scale_free=True)
        al_r = b.givens(c, s, k, sg, u_al, True,
                        dests=[std(6 + i) if i == k else None for i in range(3)])
        aa_r = b.givens(c, s, k, sg, u_aa, True)
        aa_i = list(aa_r)
        aa_i[k] = b.lin([aa_r[k], qdd], [1.0, sg], dest=std(9 + k), scale_free=True)
        ek = [0.0, 0.0, 0.0]
        ek[k] = 1.0
        al_i = list(al_r)
        for i in (a_, b_):
            bb, cc = (i + 1) % 3, (i + 2) % 3
            cva = b.lin([va_i[bb], va_i[cc]], [ek[cc], -ek[bb]])
            m1 = b.mul(cva, qd)
            aa_i[i] = b.lin([aa_r[i], m1], [1.0, sg], dest=std(9 + i),
                            scale_free=True)
            cvl = b.lin([vl_i[bb], vl_i[cc]], [ek[cc], -ek[bb]])
            m2 = b.mul(cvl, qd)
            al_i[i] = b.lin([al_r[i], m2], [1.0, sg], dest=std(6 + i),
                            scale_free=True)
        vl, va, al, aa = vl_i, va_i, al_i, aa_i

        # ---- force for this joint ----
        b.phase = f"force{j}"
        m = C["m"][j]
        mc = C["mc"][j].tolist()
        Io = C["Io"][j]
        Iv_l = [b.lin([vl[i], va[(i + 1) % 3], va[(i + 2) % 3]],
                      [m, mc[(i + 2) % 3], -mc[(i + 1) % 3]]) for i in range(3)]
        Ia_l = [b.lin([al[i], aa[(i + 1) % 3], aa[(i + 2) % 3]],
                      [m, mc[(i + 2) % 3], -mc[(i + 1) % 3]]) for i in range(3)]
        Iv_a = [b.lin([va[0], va[1], va[2], vl[(i + 2) % 3], vl[(i + 1) % 3]],
                      [Io[i][0], Io[i][1], Io[i][2],
                       mc[(i + 1) % 3], -mc[(i + 2) % 3]]) for i in range(3)]
        Ia_a = [b.lin([aa[0], aa[1], aa[2], al[(i + 2) % 3], al[(i + 1) % 3]],
                      [Io[i][0], Io[i][1], Io[i][2],
                       mc[(i + 1) % 3], -mc[(i + 2) % 3]]) for i in range(3)]
        cv1 = b.cross(va, Iv_l)
        for i in range(3):
            m1, m2 = cv1[i]
            fstore[j][i] = b.lin([Ia_l[i], m1, m2], [1.0, 1.0, -1.0],
                                 dest=b.f_ap(j, i), scale_free=True)
        cv2 = b.cross(va, Iv_a)
        cv3 = b.cross(vl, Iv_l)
        for i in range(3):
            m1, m2 = cv2[i]
            m3, m4 = cv3[i]
            fstore[j][3 + i] = b.lin([Ia_a[i], m1, m2, m3, m4],
                                     [1.0, 1.0, -1.0, 1.0, -1.0],
                                     dest=b.f_ap(j, 3 + i), scale_free=True)

    # ---- backward pass ----
    cl = [Z, Z, Z]
    ca = [Z, Z, Z]
    for j in range(D - 1, -1, -1):
        F = C["F"][j]
        p = C["p"][j]
        k, sg = C["ax"][j]
        pp = (F.T @ p).tolist()
        b.phase = f"bwd{j}"
        s, c = b.sincos(j)
        f_l = fstore[j][:3]
        f_a = fstore[j][3:]
        tl = [b.lin([f_l[i], cl[i]], [1.0, 1.0]) for i in range(3)]
        ta = [b.lin([f_a[i], ca[i]], [1.0, 1.0]) for i in range(3)]
        b.lin([ta[k], Val(b.inpc("qd", j))], [sg, C["damping"][j]],
              dest=b.out_ap(j))
        if j == 0:
            continue
        w_l = b.givens(c, s, k, sg, tl, False)
        w_a = b.givens(c, s, k, sg, ta, False)
        x = []
        for i in range(3):
            bb, cc = (i + 1) % 3, (i + 2) % 3
            x.append(b.lin([w_a[i], w_l[cc], w_l[bb]], [1.0, pp[bb], -pp[cc]]))
        cl = b.matvec(F.tolist(), w_l)
        ca = b.matvec(F.tolist(), x)


# ---------------------------------------------------------------------------
# numpy backend (validation)
# ---------------------------------------------------------------------------
class NumpyBuilder(Builder):
    def __init__(self, q, qd, qdd, f16=False):
        super().__init__()
        self.q, self.qd, self.qdd = q, qd, qdd
        self.N = q.shape[0]
        self.out = np.zeros((self.N, D), np.float32)
        self._f = {}
        self.f16 = f16

    def _w(self, r, dest):
        if self.f16:
            r = r.astype(np.float16).astype(np.float32)
        if dest is not None:
            dest[...] = r
            return dest
        return r

    def _f32(self, x):
        return np.asarray(x, np.float32)

    def p_stt(self, in0, scalar, in1, op1, dest=None):
        r = self._f32(in0 * np.float32(scalar))
        if op1 == "add":
            r = self._f32(r + in1)
        elif op1 == "subtract":
            r = self._f32(r - in1)
        else:
            r = self._f32(r * in1)
        return self._w(r, dest)

    def p_tt(self, in0, in1, op, dest=None):
        if op == "mult":
            r = self._f32(in0 * in1)
        elif op == "add":
            r = self._f32(in0 + in1)
        else:
            r = self._f32(in0 - in1)
        return self._w(r, dest)

    def p_affine(self, in0, scale, bias, dest=None):
        return self._w(self._f32(in0 * np.float32(scale) + np.float32(bias)), dest)

    def p_sinact(self, in0, scale, bias):
        r = self._f32(np.sin(self._f32(in0 * np.float32(scale) + np.float32(bias))))
        if self.f16:
            r = r.astype(np.float16).astype(np.float32)
        return r

    def p_ones(self):
        return np.ones(self.N, np.float32)

    def inp(self, name, j):
        return {"q": self.q, "qd": self.qd, "qdd": self.qdd}[name][:, j].astype(
            np.float32
        )

    def inpc(self, name, j):
        v = self.inp(name, j)
        if self.f16:
            v = v.astype(np.float16).astype(np.float32)
        return v

    def sincos(self, j):
        # numeric equivalent of Builder.sincos with the fp32 reduction kept
        # unrounded (the magic constant overflows a naive f16 emulation)
        self.n_trig += 2
        x = self.inp("q", j)
        TWO_PI = np.float32(2.0 * math.pi)
        r = np.round((x / TWO_PI).astype(np.float32))
        z = (x - TWO_PI * r).astype(np.float32)

        def rnd(v):
            v = np.asarray(v, np.float32)
            return v.astype(np.float16).astype(np.float32) if self.f16 else v

        spl = rnd(np.sin(z))
        sh = rnd(np.sin(np.float32(0.5) * z))
        sq = rnd(sh * sh)
        cpl = rnd(np.float32(1.0) - np.float32(2.0) * sq)
        return Val(spl, 1.0, 0.0), Val(cpl, 1.0, 0.0)

    def out_ap(self, j):
        return self.out[:, j]

    def f_ap(self, j, i):
        key = (j, i)
        if key not in self._f:
            self._f[key] = np.empty(self.N, np.float32)
        return self._f[key]

    def state_ap(self, j, i):
        return np.empty(self.N, np.float32)


def rnea_numpy(q, qd, qdd, rot_fix, trans_fix, joint_axes, mass, com, inertia,
               damping):
    C = host_consts(rot_fix, trans_fix, joint_axes, mass, com, inertia, damping)
    b = NumpyBuilder(q, qd, qdd)
    build_rnea(b, C)
    return b.out


# ---------------------------------------------------------------------------
# IR backend: records ops on integer-token planes
# ---------------------------------------------------------------------------
class IRBuilder(Builder):
    def __init__(self):
        super().__init__()
        self.ops = []   # (kind, out_token, in_tokens, params, phase)
        self._n = 0
        self.phase = ""
        self.f32_toks = set()   # tmp tokens that must stay fp32 (trig chain)
        self._casts = {}

    def _tmp(self):
        self._n += 1
        return ("t", self._n)

    def plane_key(self, pl):
        return pl

    def same_plane(self, a, b):
        return a == b

    def p_stt(self, in0, scalar, in1, op1, dest=None):
        out = dest if dest is not None else self._tmp()
        self.ops.append(("stt", out, (in0, in1), (float(scalar), op1),
                         self.phase))
        return out

    def p_tt(self, in0, in1, op, dest=None):
        out = dest if dest is not None else self._tmp()
        self.ops.append(("tt", out, (in0, in1), (op,), self.phase))
        return out

    def p_affine(self, in0, scale, bias, dest=None):
        out = dest if dest is not None else self._tmp()
        self.ops.append(("affine", out, (in0,), (float(scale), float(bias)),
                         self.phase))
        return out

    def p_sinact(self, in0, scale, bias):
        out = self._tmp()
        self.ops.append(("sinact", out, (in0,), (float(bias), float(scale)),
                         self.phase))
        return out

    def mark_f32(self, toks):
        self.f32_toks.update(toks)

    def p_ones(self):
        out = ("ones",)
        self.ops.append(("memset", out, (), (1.0,), self.phase))
        return out

    def inp(self, name, j):
        return ("in", name, j)

    def inpc(self, name, j):
        key = (name, j)
        if key not in self._casts:
            self._casts[key] = self.p_affine(("in", name, j), 1.0, 0.0)
        return self._casts[key]

    def out_ap(self, j):
        return ("out", j)

    def f_ap(self, j, i):
        return ("f", j, i)

    def state_ap(self, j, i):
        return self._tmp()


def dce(ops):
    """drop ops whose results are never used (named 'out'/'f' sinks are live;
    'f' only if read)."""
    needed = set()
    keep = [False] * len(ops)
    for idx in range(len(ops) - 1, -1, -1):
        kind, out, ins, params, phase = ops[idx]
        if out[0] == "out" or out in needed:
            keep[idx] = True
            for t in ins:
                needed.add(t)
    return [op for k2, op in zip(keep, ops) if k2]


def ir_stats(ops):
    from collections import Counter

    c = Counter(k for k, *_ in ops)
    # liveness: peak concurrent tmp planes
    last_use = {}
    for idx, (kind, out, ins, params, phase) in enumerate(ops):
        for t in ins:
            if t[0] == "t":
                last_use[t] = idx
    live = set()
    peak = 0
    for idx, (kind, out, ins, params, phase) in enumerate(ops):
        if out[0] == "t":
            live.add(out)
        peak = max(peak, len(live))
        for t in ins:
            if t[0] == "t" and last_use.get(t) == idx:
                live.discard(t)
    return dict(c), peak


# ---------------------------------------------------------------------------
# engine assignment: HEFT-style list scheduling over DVE / Pool / ACT
# ---------------------------------------------------------------------------
# measured per-op costs on [128, 512] tiles (TimelineSim == HW +-4%).
# HW ISA facts: Pool supports tensor_tensor / tensor_scalar-imm / copy but NOT
# scalar_tensor_tensor; DVE f16 fast modes need every operand 2-byte.
ENGINE_FILTER = {"DVE", "ACT"}   # HW-raced: DVE + a capped ACT dose;
                                 # POOL and uncapped ACT lose on HW
ACT_TRIG_ONLY = False  # restrict ACT to the trig pipeline (plus sinact)
ACT_MAX_OPS = 600      # cap on ACT-assigned affine ops (edge-dose control;
                       # HW dose-response flat 200-500, sim prefers more)
POOL_PHASES = ()       # phases owned exclusively by the Pool engine
XENG_NS = 260.0       # cross-engine dependency (semaphore) latency
EDGE_COST_NS = 0.0    # busy-time surcharge per cross-engine input (makes the
                      # greedy actively avoid creating semaphore edges)
RAW_BUBBLE_NS = 120.0  # extra delay when consumer directly follows producer
                      # on the same engine (SBUF write->read turnaround)
SCHED_WINDOW = 380    # max scheduler runahead past the frontier
SCHED_PRESSURE = 56   # live-tmp count above which pressure relief kicks in


def schedule_engines(ops, f32_toks=frozenset(), dtype16=True):
    """Assign each op an engine and a global emission order. Returns
    (ordered ops, engine per ordered op, est. makespan ns)."""

    def is16(t):
        if t[0] == "t":
            return dtype16 and t not in f32_toks
        if t[0] == "f":
            return True
        if t[0] == "ones":
            return dtype16
        return False   # in / out dram-backed chunks are fp32

    def op_costs(i):
        kind, out, ins, params, phase = ops[i]
        fast = dtype16 and is16(out) and all(is16(t) for t in ins)
        if kind == "stt":
            c = {"DVE": 594.0}
        elif kind == "tt":
            c = {"DVE": 327.0 if fast else 594.0, "POOL": 1111.0}
            if phase in POOL_PHASES:
                return {"POOL": 1111.0}
        elif kind == "affine":
            c = {"DVE": 148.0 if fast else 297.5, "ACT": 612.0,
                 "POOL": 806.0}
            if phase in POOL_PHASES:
                return {"POOL": 806.0}
            if ACT_TRIG_ONLY and out not in f32_toks:
                c.pop("ACT")
        elif kind == "sinact":
            return {"ACT": 612.0}
        elif kind == "memset":
            return {"DVE": 297.5}
        else:
            raise ValueError(kind)
        c2 = {e: v for e, v in c.items() if e in ENGINE_FILTER}
        return c2 or c

    n = len(ops)
    prod = {}
    for i, (kind, out, ins, params, phase) in enumerate(ops):
        prod[out] = i
    preds = [[] for _ in range(n)]
    succs = [[] for _ in range(n)]
    for i, (kind, out, ins, params, phase) in enumerate(ops):
        seen = set()
        for t in ins:
            j = prod.get(t)
            if j is not None and j < i and j not in seen:
                seen.add(j)
                preds[i].append(j)
                succs[j].append(i)
    costs = [op_costs(i) for i in range(n)]
    mincost = [min(costs[i].values()) for i in range(n)]
    rank = [0.0] * n
    for i in range(n - 1, -1, -1):
        m = 0.0
        for s in succs[i]:
            if rank[s] > m:
                m = rank[s]
        rank[i] = mincost[i] + m
    # register pressure bookkeeping: how many pending reads of each tmp
    uses = {}
    for i, (kind, out, ins, params, phase) in enumerate(ops):
        for t in ins:
            if t[0] == "t":
                uses[t] = uses.get(t, 0) + 1

    def presses(i):
        """net change in live tmp count if op i runs now."""
        kind, out, ins, params, phase = ops[i]
        d = 1 if (out[0] == "t") else 0
        for t in set(ins):
            if t[0] == "t" and uses[t] == sum(1 for x in ins if x == t):
                d -= 1
        return d

    ndeps = [len(preds[i]) for i in range(n)]
    ready = [i for i in range(n) if ndeps[i] == 0]
    eng_free = {"DVE": 0.0, "POOL": 0.0, "ACT": 0.0}
    eng_last = {"DVE": -1, "POOL": -1, "ACT": -1}   # last op on each engine
    act_used = 0
    finish = [0.0] * n
    start = [0.0] * n
    eng_of = [None] * n
    order = []
    live = 0
    done = 0
    scheduled = [False] * n
    frontier = 0          # lowest unscheduled original index
    WINDOW = SCHED_WINDOW
    PRESSURE_HI = SCHED_PRESSURE
    while ready:
        while frontier < n and scheduled[frontier]:
            frontier += 1
        elig = [x for x in ready if x < frontier + WINDOW]
        if not elig:
            elig = ready
        if live > PRESSURE_HI:
            i = min(elig, key=lambda x: (presses(x), -rank[x]))
        else:
            i = max(elig, key=lambda x: rank[x])
        ready.remove(i)
        scheduled[i] = True
        kind = ops[i][0]
        best = None
        cand = costs[i]
        if kind == "affine" and act_used >= ACT_MAX_OPS and "ACT" in cand \
                and len(cand) > 1:
            cand = {e: v for e, v in cand.items() if e != "ACT"}
        for e, c in cand.items():
            est = eng_free[e]
            n_cross = 0
            for p in preds[i]:
                if eng_of[p] != e:
                    d = finish[p] + XENG_NS
                    n_cross += 1
                elif p == eng_last[e]:
                    d = finish[p] + RAW_BUBBLE_NS
                else:
                    d = finish[p]
                if d > est:
                    est = d
            f = est + c + EDGE_COST_NS * n_cross
            if best is None or f < best[0] or (f == best[0] and c < best[1]):
                best = (f, c, e, est)
        f, c, e, st = best
        eng_of[i] = e
        start[i] = st
        finish[i] = f
        eng_free[e] = f
        eng_last[e] = i
        if e == "ACT" and kind == "affine":
            act_used += 1
        done += 1
        kind_i, out_i, ins_i, _, _ = ops[i]
        if out_i[0] == "t":
            live += 1
        for t in ins_i:
            if t[0] == "t":
                uses[t] -= 1
                if uses[t] == 0:
                    live -= 1
        for s in succs[i]:
            ndeps[s] -= 1
            if ndeps[s] == 0:
                ready.append(s)
        order.append(i)
    assert done == n
    return ([ops[i] for i in order], [eng_of[i] for i in order],
            max(finish) if n else 0.0)


def build_ir(C):
    b = IRBuilder()
    build_rnea(b, C)
    ops = dce(b.ops)
    return ops, b


# ---------------------------------------------------------------------------
# bass emission from IR
# ---------------------------------------------------------------------------
def emit_bass(nc, tc, pools, chunks, out_chunk, ops, engines, f32_toks,
              fd=FD, bench_alias_out=False, dtype16=True, fstore16=True):
    from concourse import mybir

    f32 = mybir.dt.float32
    fdt = mybir.dt.float16 if dtype16 else mybir.dt.float32
    fsdt = mybir.dt.float16 if (fstore16 or dtype16) else mybir.dt.float32
    ALU = {"add": mybir.AluOpType.add, "subtract": mybir.AluOpType.subtract,
           "mult": mybir.AluOpType.mult}

    def tok_dt(tok):
        if tok[0] == "t" and tok in f32_toks:
            return f32
        return fdt

    last_use = {}
    for idx, (kind, out, ins, params, phase) in enumerate(ops):
        for t in ins:
            if t[0] == "t":
                last_use[t] = idx

    from collections import deque

    ftiles = {}
    tmp_ap = {}         # token -> AP
    reg_of = {}         # token -> (dtype key, reg index)
    free_regs = {"f32": deque(), "f16": deque()}  # FIFO: max reuse distance
    n_regs = {"f32": 0, "f16": 0}
    serial = 0

    def named_ap(tok):
        nonlocal serial
        if tok[0] == "in":
            _, name, j = tok
            v = chunks[name].rearrange("p (f d) -> p d f", d=D)
            return v[:, j, :]
        if tok[0] == "out":
            base = chunks["qdd"] if bench_alias_out else out_chunk
            v = base.rearrange("p (f d) -> p d f", d=D)
            return v[:, tok[1], :]
        if tok[0] == "f":
            _, j, i = tok
            if j not in ftiles:
                serial += 1
                ftiles[j] = pools["fst"].tile([P, 6 * fd], fsdt, tag=f"f{j}",
                                              name=f"f{j}", bufs=1)
            t = ftiles[j]
            return t[:, i * fd:(i + 1) * fd]
        if tok[0] == "ones":
            return ones_ap
        raise KeyError(tok)

    def get_ap(tok):
        if tok[0] == "t":
            return tmp_ap[tok]
        return named_ap(tok)

    def alloc_out(tok, idx, eng):
        nonlocal serial
        if tok[0] != "t":
            return named_ap(tok)
        dt = tok_dt(tok)
        key = "f32" if dt == f32 else "f16"
        fr = free_regs[key]
        r = None
        if fr:
            # prefer a register whose WAR predecessor (last reader) ran on
            # the same engine: the hazard is then free (program order)
            for k in range(len(fr)):
                if fr[k][1] == eng:
                    r = fr[k][0]
                    del fr[k]
                    break
            if r is None:
                r = fr.popleft()[0]
        else:
            r = n_regs[key]
            n_regs[key] += 1
        reg_of[tok] = (key, r)
        serial += 1
        t = pools["reg"].tile([P, fd], dt, tag=f"r{key}_{r}",
                              name=f"v{serial}", bufs=1)
        tmp_ap[tok] = t[:, :]
        return tmp_ap[tok]

    def release_ins(ins, idx, eng):
        for t in ins:
            if t[0] == "t" and last_use.get(t) == idx:
                kr = reg_of.pop(t, None)
                if kr is not None:
                    free_regs[kr[0]].append((kr[1], eng))

    ones_ap = None
    n_eng = {"DVE": 0, "POOL": 0, "ACT": 0}
    Copy = mybir.ActivationFunctionType.Copy
    for idx, (kind, out, ins, params, phase) in enumerate(ops):
        eng = engines[idx]
        n_eng[eng] += 1
        if kind == "memset":
            serial += 1
            t = pools["misc"].tile([P, fd], fdt, tag="ones", name="ones", bufs=1)
            ones_ap = t[:, :]
            nc.vector.memset(ones_ap, 1.0)
            continue
        out_ap = alloc_out(out, idx, eng)
        if kind == "stt":
            scalar, op1 = params
            assert eng == "DVE"   # Pool has no scalar_tensor_tensor on TRN2
            nc.vector.scalar_tensor_tensor(out_ap, get_ap(ins[0]), scalar,
                                           get_ap(ins[1]),
                                           mybir.AluOpType.mult, ALU[op1])
        elif kind == "tt":
            e = nc.vector if eng == "DVE" else nc.gpsimd
            e.tensor_tensor(out_ap, get_ap(ins[0]), get_ap(ins[1]),
                            ALU[params[0]])
        elif kind == "affine":
            scale, bias = params
            if eng == "ACT":
                nc.scalar.activation(out_ap, get_ap(ins[0]), Copy,
                                     bias=bias, scale=scale)
            elif eng == "DVE":
                nc.vector.tensor_scalar(out_ap, get_ap(ins[0]), scale, bias,
                                        mybir.AluOpType.mult,
                                        mybir.AluOpType.add)
            else:
                nc.gpsimd.tensor_scalar(out_ap, get_ap(ins[0]), scale, bias,
                                        mybir.AluOpType.mult,
                                        mybir.AluOpType.add)
        elif kind == "sinact":
            bias, scale = params
            nc.scalar.activation(out_ap, get_ap(ins[0]),
                                 mybir.ActivationFunctionType.Sin,
                                 bias=bias, scale=scale)
        else:
            raise ValueError(kind)
        release_ins(ins, idx, eng)
    return n_regs, n_eng


def _build_nc(C, verbose=False, repeat=1, dtype16=True):
    import concourse.bacc as bacc
    import concourse.tile as tile_mod
    from concourse import mybir

    ops, bstat = build_ir(C)
    f32_toks = bstat.f32_toks
    ops, engines, est = schedule_engines(ops, f32_toks=f32_toks,
                                         dtype16=dtype16)
    if verbose:
        stats, peak = ir_stats(ops)
        print("IR ops:", stats, "peak live tmps:", peak)
        print("schedule est makespan: %.0f ns" % est)

    nc = bacc.Bacc()
    f32 = mybir.dt.float32
    # register pi/2 as a const AP (Sin activation bias must be a const AP)
    halfpi = float(math.pi / 2)
    _ct = nc.alloc_sbuf_tensor("const-f32-halfpi", [128, 1], f32)
    nc.gpsimd.memset(_ct.ap(), halfpi)
    nc.const_aps.aps[(f32, halfpi)] = _ct.ap()
    nc.all_engine_barrier()
    q_d = nc.dram_tensor("q", [SHARD, D], f32, kind="ExternalInput")
    qd_d = nc.dram_tensor("qd", [SHARD, D], f32, kind="ExternalInput")
    qdd_d = nc.dram_tensor("qdd", [SHARD, D], f32, kind="ExternalInput")
    tau_d = nc.dram_tensor("tau", [SHARD, D], f32, kind="ExternalOutput")

    with ExitStack() as ctx:
        tc = ctx.enter_context(tile_mod.TileContext(nc))
        io_pool = ctx.enter_context(tc.tile_pool(name="io", bufs=1))
        fst_pool = ctx.enter_context(tc.tile_pool(name="fst", bufs=1))
        reg_pool = ctx.enter_context(tc.tile_pool(name="reg", bufs=1))
        misc_pool = ctx.enter_context(tc.tile_pool(name="misc", bufs=1))
        pools = {"io": io_pool, "fst": fst_pool, "reg": reg_pool,
                 "misc": misc_pool}

        chunks = {}
        for name, dram in (("q", q_d), ("qd", qd_d), ("qdd", qdd_d)):
            t = io_pool.tile([P, D * FD], f32, tag=f"io_{name}",
                             name=f"ch_{name}", bufs=1)
            nc.sync.dma_start(t[:, :],
                              dram[:, :].rearrange("(p f) d -> p (f d)", p=P))
            chunks[name] = t

        if repeat == 1:
            # out chunk shares the qdd slot (qdd is fully consumed by the
            # forward pass before any tau is written)
            out_chunk = io_pool.tile([P, D * FD], f32, tag="io_qdd",
                                     name="ch_out", bufs=1)
            n_regs, n_eng = emit_bass(nc, tc, pools, chunks, out_chunk, ops,
                                      engines, f32_toks, dtype16=dtype16)
        else:
            # bench mode: dedicated out tile (aliasing qdd would serialize
            # body k+1's forward pass behind body k's tau writes)
            out_chunk = io_pool.tile([P, D * FD], f32, tag="io_out",
                                     name="ch_out", bufs=1)
            for _ in range(repeat):
                n_regs, n_eng = emit_bass(nc, tc, pools, chunks, out_chunk,
                                          ops, engines, f32_toks,
                                          dtype16=dtype16)
        if verbose:
            print("registers used:", n_regs, "engine mix:", n_eng)

        nc.sync.dma_start(tau_d[:, :].rearrange("(p f) d -> p (f d)", p=P),
                          out_chunk[:, :])
    if not nc.is_finalized():
        nc.finalize()
    return nc


def kernel(**inputs):
    q = np.ascontiguousarray(inputs["q"], np.float32)
    qd = np.ascontiguousarray(inputs["qd"], np.float32)
    qdd = np.ascontiguousarray(inputs["qdd_des"], np.float32)
    C = host_consts(inputs["rot_fix"], inputs["trans_fix"], inputs["joint_axes"],
                    inputs["mass"], inputs["com"], inputs["inertia"],
                    inputs["damping"])
    nc = _build_nc(C)

    from concourse.bass_utils import run_bass_kernel_spmd

    in_maps = []
    for i in range(N_CORES):
        sl = slice(i * SHARD, (i + 1) * SHARD)
        in_maps.append({"q": q[sl], "qd": qd[sl], "qdd": qdd[sl]})
    res = run_bass_kernel_spmd(nc, in_maps, list(range(N_CORES)))
    out = np.concatenate([res.results[i]["tau"] for i in range(N_CORES)], 0)
    return out.astype(np.float32)

